# revision 1
# baseline (speedup 1.0000x reference)
"""Trainium2 Bass kernel for nn_CDTripletLoss (segment_reduce).

Strategy: community-sharded data layout. Host sorts nodes by community,
pads each community to 256 slots, assigns 64 communities per core.
Device computes per-community stats (bn_stats), one AllReduce of the
[512,128] community-sum matrix, then per-node distances via bf16 matmuls
([node,comm] tiles), sqrt on ScalarE, row-sum + masked min on VectorE,
and the triplet/std losses via per-node column algebra. Host combines the
8 cores' partial sums.
"""
import numpy as np
import ml_dtypes

import concourse.bass as bass
import concourse.tile as tile
from concourse import bacc, mybir
from concourse.bass_utils import run_bass_kernel_spmd

f32 = mybir.dt.float32
bf16 = mybir.dt.bfloat16
fp16 = mybir.dt.float16
AX = mybir.AxisListType
OP = mybir.AluOpType
ACTF = mybir.ActivationFunctionType

NCORES = 8
C = 512            # communities
CPC = 64           # communities per core
KSLOT = 256        # padded slots per community
SLOTS = CPC * KSLOT   # 16384 slots per core
D = 128
NT = SLOTS // 128  # 128 node tiles per core
GRP = 4            # tiles per dist group
NGRP = NT // GRP
N_NODES = 100000
ALPHA = 0.25
P_EPS = 1e-6
BIGM = 1.0e6
BIG16 = 60000.0

_PROG = None
_NO_CC = False     # timing-only: replace the collective with a local DMA
_REPEAT_C = 1      # timing-only: repeat phase C to isolate its cost
_VARIANT = "optA"  # "v1" (ACT accum sum, fp32 min) or "optA" (batched ACT, fp16 min)


def _build_program():
    nc = bacc.Bacc("TRN2", target_bir_lowering=False, debug=False, num_devices=NCORES)

    xT_in = nc.declare_dram_parameter("xT", [D, SLOTS], bf16, isOutput=False)
    x2c_in = nc.declare_dram_parameter("x2c", [128, NT], f32, isOutput=False)
    valid_in = nc.declare_dram_parameter("valid", [128, NT], f32, isOutput=False)
    ntcr_in = nc.declare_dram_parameter("ntcr", [1, C], f32, isOutput=False)   # -2/cnt rotated
    cntc_in = nc.declare_dram_parameter("cntc", [128, NT], f32, isOutput=False)
    acol_in = nc.declare_dram_parameter("acol", [128, NT], f32, isOutput=False)
    cnt2c_in = nc.declare_dram_parameter("cnt2c", [128, NT], f32, isOutput=False)
    cntrow_in = nc.declare_dram_parameter("cntrow", [1, CPC], f32, isOutput=False)
    cm1r_in = nc.declare_dram_parameter("cm1r", [1, CPC], f32, isOutput=False)
    pplace_in = nc.declare_dram_parameter("pplace", [CPC, C], f32, isOutput=False)
    prot_in = nc.declare_dram_parameter("prot", [C, C], f32, isOutput=False)
    ident_in = nc.declare_dram_parameter("ident", [128, 128], f32, isOutput=False)
    if _VARIANT == "optA":
        x2r_in = nc.declare_dram_parameter("x2r", [2, SLOTS], bf16, isOutput=False)
        nsx_in = nc.declare_dram_parameter("nsx", [128, NT], f32, isOutput=False)

    out_d = nc.declare_dram_parameter("out", [4, 1], f32, isOutput=True)
    dbg_d = nc.declare_dram_parameter("dbg", [128, 8], f32, isOutput=True)

    with tile.TileContext(nc, num_cores=NCORES) as tc:
        with (
            tc.tile_pool(name="pers", bufs=1) as pers,
            tc.tile_pool(name="dist", bufs=3) as distp,
            tc.tile_pool(name="dram", bufs=1, space="DRAM") as dramp,
        ):
            # ---------- Phase 0: loads ----------
            xT = pers.tile([D, SLOTS], bf16)
            for ch in range(8):
                nc.gpsimd.dma_start(xT[:, 2048 * ch : 2048 * (ch + 1)],
                                    xT_in[:, 2048 * ch : 2048 * (ch + 1)])
            x2c = pers.tile([128, NT], f32)
            nc.gpsimd.dma_start(x2c[:], x2c_in[:])
            valid = pers.tile([128, NT], f32)
            nc.gpsimd.dma_start(valid[:], valid_in[:])
            ntcr = pers.tile([1, C], f32)
            nc.gpsimd.dma_start(ntcr[:], ntcr_in[:])
            cntc = pers.tile([128, NT], f32)
            nc.gpsimd.dma_start(cntc[:], cntc_in[:])
            acol = pers.tile([128, NT], f32)
            nc.gpsimd.dma_start(acol[:], acol_in[:])
            cnt2c = pers.tile([128, NT], f32)
            nc.gpsimd.dma_start(cnt2c[:], cnt2c_in[:])
            cntrow = pers.tile([1, CPC], f32)
            nc.gpsimd.dma_start(cntrow[:], cntrow_in[:])
            cm1r = pers.tile([1, CPC], f32)
            nc.gpsimd.dma_start(cm1r[:], cm1r_in[:])
            pplace = pers.tile([CPC, C], f32)
            nc.gpsimd.dma_start(pplace[:], pplace_in[:])
            prot = pers.tile([128, 4, C], f32)
            nc.gpsimd.dma_start(prot[:], prot_in.rearrange("(ch c) f -> c ch f", ch=4))
            ident = pers.tile([128, 128], f32)
            nc.gpsimd.dma_start(ident[:], ident_in[:])
            if _VARIANT == "optA":
                aug4 = pers.tile([4, SLOTS], bf16)
                nc.vector.memset(aug4[0:2, :], 1.0)
                nc.gpsimd.dma_start(aug4[2:4, :], x2r_in[:])
                nsx = pers.tile([128, NT], f32)
                nc.gpsimd.dma_start(nsx[:], nsx_in[:])
                onesrow_b = pers.tile([1, C], bf16)
                nc.vector.memset(onesrow_b[:], 1.0)

            ones1 = pers.tile([1, 128], f32)
            nc.vector.memset(ones1[:], 1.0)
            ones2b = pers.tile([2, 128], bf16)
            nc.vector.memset(ones2b[:], 1.0)
            onescol = pers.tile([128, 1], f32)
            nc.vector.memset(onescol[:], 1.0)

            # ---------- Phase A: per-community stats ----------
            bnb = pers.tile([128, CPC, 6], f32)
            for j in range(CPC):
                nc.vector.bn_stats(bnb[:, j, :], xT[:, KSLOT * j : KSLOT * (j + 1)])
            mA = bnb[:, :, 1]
            m2A = bnb[:, :, 2]
            mB = bnb[:, :, 4]
            m2B = bnb[:, :, 5]
            Sloc = pers.tile([128, CPC], f32)
            nc.vector.tensor_add(Sloc[:], mA, mB)
            nc.vector.tensor_scalar(Sloc[:], Sloc[:], 128.0, None, op0=OP.mult)
            SQloc = pers.tile([128, CPC], f32)
            tA = pers.tile([128, CPC], f32)
            nc.vector.tensor_mul(tA[:], mA, mA)
            tB = pers.tile([128, CPC], f32)
            nc.vector.tensor_mul(tB[:], mB, mB)
            nc.vector.tensor_add(tA[:], tA[:], tB[:])
            nc.vector.tensor_scalar(tA[:], tA[:], 128.0, None, op0=OP.mult)
            nc.vector.tensor_add(SQloc[:], m2A, m2B)
            nc.vector.tensor_add(SQloc[:], SQloc[:], tA[:])

            # ---------- Phase B: collective + means prep ----------
            with tc.tile_pool(name="psB", bufs=2, space="PSUM") as psB:
                ps_t = psB.tile([CPC, 128], f32, tag="ps")
                nc.tensor.transpose(ps_t[:], Sloc[:], ident[:])
                S_jd = pers.tile([CPC, 128], f32)
                nc.vector.tensor_copy(S_jd[:], ps_t[:])
                ps_g = psB.tile([128, 4, 128], f32, tag="ps")
                for ch in range(4):
                    nc.tensor.matmul(ps_g[:, ch, :], pplace[:, 128 * ch : 128 * (ch + 1)],
                                     S_jd[:], start=True, stop=True)
                sg = pers.tile([128, 4, 128], f32)
                nc.vector.tensor_copy(sg[:], ps_g[:])
                ccin = dramp.tile([C, D], f32)
                nc.gpsimd.dma_start(ccin.rearrange("(ch c) d -> c ch d", ch=4), sg[:])
                ccout = dramp.tile([C, D], f32)
                if _NO_CC:
                    nc.gpsimd.dma_start(ccout[:], ccin[:])
                else:
                    nc.gpsimd.collective_compute(
                        "AllReduce", OP.add,
                        replica_groups=[list(range(NCORES))],
                        ins=[ccin[:].opt()],
                        outs=[ccout[:].opt()],
                    )
                gsb = pers.tile([128, 4, 128], f32)
                nc.gpsimd.dma_start(gsb[:], ccout.rearrange("(ch c) d -> c ch d", ch=4))
                ps_r = psB.tile([128, C], f32, tag="ps")
                for ch in range(4):
                    nc.tensor.matmul(ps_r[:], gsb[:, ch, :], prot[:, ch, :],
                                     start=(ch == 0), stop=(ch == 3))
                Srot = pers.tile([128, C], f32)
                nc.vector.tensor_copy(Srot[:], ps_r[:])
                ps_b = psB.tile([128, C], f32, tag="ps")
                nc.tensor.matmul(ps_b[:], ones1[:], ntcr[:], start=True, stop=True)
                m2T = pers.tile([128, C], bf16)
                nc.vector.tensor_mul(m2T[:], Srot[:], ps_b[:])
                msq = pers.tile([128, C], f32)
                nc.vector.tensor_mul(msq[:], m2T[:], m2T[:])
                ps_c = psB.tile([1, C], f32, tag="ps")
                nc.tensor.matmul(ps_c[:], onescol[:], msq[:], start=True, stop=True)
                c2f = pers.tile([1, C], f32)
                nc.vector.tensor_scalar(c2f[:], ps_c[:], 0.25, None, op0=OP.mult)
                c2hl = pers.tile([2, C], bf16)
                nc.vector.tensor_copy(c2hl[0:1, :], c2f[:])
                c2lo = pers.tile([1, C], f32)
                nc.vector.tensor_sub(c2lo[:], c2f[:], c2hl[0:1, :])
                c2lob = pers.tile([1, C], bf16)
                nc.vector.tensor_copy(c2lob[:], c2lo[:])
                nc.gpsimd.dma_start(c2hl[1:2, :], c2lob[:])
                if _VARIANT == "optA":
                    # rhs for MM2: [c2hi; c2lo; ones; ones]
                    c2x2r = pers.tile([4, C], bf16)
                    nc.gpsimd.dma_start(c2x2r[0:2, :], c2hl[:])
                    nc.gpsimd.dma_start(c2x2r[2:3, :], onesrow_b[:])
                    nc.gpsimd.dma_start(c2x2r[3:4, :], onesrow_b[:])
                c2rep = pers.tile([1, NT], f32)
                nc.vector.tensor_copy(c2rep[0:1, 0:NT:2], c2f[0:1, 0:CPC])
                nc.vector.tensor_copy(c2rep[0:1, 1:NT:2], c2f[0:1, 0:CPC])
                ps_cc = psB.tile([128, NT], f32, tag="ps")
                nc.tensor.matmul(ps_cc[:], ones1[:], c2rep[:], start=True, stop=True)
                c2cols = pers.tile([128, NT], f32)
                nc.vector.tensor_copy(c2cols[:], ps_cc[:])
                ps_cb = psB.tile([128, CPC], f32, tag="ps")
                nc.tensor.matmul(ps_cb[:], ones1[:], cntrow[:], start=True, stop=True)
                cntbc = pers.tile([128, CPC], f32)
                nc.vector.tensor_copy(cntbc[:], ps_cb[:])
                ps_cb2 = psB.tile([128, CPC], f32, tag="ps")
                nc.tensor.matmul(ps_cb2[:], ones1[:], cm1r[:], start=True, stop=True)
                cm1bc = pers.tile([128, CPC], f32)
                nc.vector.tensor_copy(cm1bc[:], ps_cb2[:])

            # ---------- Phase C: distance tiles ----------
            sumc = pers.tile([128, NT], f32)
            ownc = pers.tile([128, NT], f32)
            minc = pers.tile([128, NT], f32)
            if _VARIANT == "optA":
                min16 = pers.tile([128, NT], fp16)
            with tc.tile_pool(name="psC", bufs=2 if _VARIANT == "optA" else 8,
                              space="PSUM") as psC:
                for _rep in range(_REPEAT_C):
                    if _VARIANT == "v1":
                        for g in range(NGRP):
                            dg = distp.tile([128, GRP * C], f32)
                            for q in range(GRP):
                                t = GRP * g + q
                                ps = psC.tile([128, C], f32, tag="ps")
                                nc.tensor.matmul(ps[:], xT[:, 128 * t : 128 * (t + 1)],
                                                 m2T[:], start=True, stop=False)
                                nc.tensor.matmul(ps[:], ones2b[:], c2hl[:],
                                                 start=False, stop=True)
                                nc.scalar.activation(dg[:, C * q : C * (q + 1)], ps[:],
                                                     ACTF.Sqrt,
                                                     bias=x2c[:, t : t + 1], scale=1.0,
                                                     accum_out=sumc[:, t : t + 1])
                                oc = C * q + (t // 2)
                                nc.vector.tensor_copy(ownc[:, t : t + 1], dg[:, oc : oc + 1])
                                nc.vector.memset(dg[:, oc : oc + 1], BIGM)
                            nc.vector.tensor_reduce(
                                minc[:, GRP * g : GRP * (g + 1)],
                                dg.rearrange("p (q c) -> p q c", q=GRP),
                                axis=AX.X, op=OP.min,
                            )
                    else:
                        for g in range(NGRP):
                            dg = distp.tile([128, GRP * C], f32)
                            d16 = distp.tile([128, GRP * C], fp16)
                            ps4 = psC.tile([128, GRP * C], f32, tag="ps")
                            for q in range(GRP):
                                t = GRP * g + q
                                nc.tensor.matmul(ps4[:, C * q : C * (q + 1)],
                                                 xT[:, 128 * t : 128 * (t + 1)],
                                                 m2T[:], start=True, stop=False)
                                nc.tensor.matmul(ps4[:, C * q : C * (q + 1)],
                                                 aug4[:, 128 * t : 128 * (t + 1)],
                                                 c2x2r[:], start=False, stop=True)
                            nc.scalar.activation(dg[:], ps4[:], ACTF.Sqrt)
                            for q in range(GRP):
                                t = GRP * g + q
                                nc.vector.tensor_scalar(
                                    d16[:, C * q : C * (q + 1)], dg[:, C * q : C * (q + 1)],
                                    nsx[:, t : t + 1], 0.0, op0=OP.add, op1=OP.add,
                                    accum_out=sumc[:, t : t + 1])
                            # own cols of tiles 4g..4g+3 sit at stride-C pairs:
                            # [2g, C+2g] and [2C+2g+1, 3C+2g+1]
                            o0 = 2 * g
                            o1 = 2 * C + 2 * g + 1
                            nc.vector.tensor_copy(
                                ownc[:, 4 * g : 4 * g + 2],
                                dg[:, o0 : o0 + C + 1 : C])
                            nc.vector.tensor_copy(
                                ownc[:, 4 * g + 2 : 4 * g + 4],
                                dg[:, o1 : o1 + C + 1 : C])
                            nc.vector.memset(d16[:, o0 : o0 + C + 1 : C], BIG16)
                            nc.vector.memset(d16[:, o1 : o1 + C + 1 : C], BIG16)
                            nc.vector.tensor_reduce(
                                min16[:, GRP * g : GRP * (g + 1)],
                                d16.rearrange("p (q c) -> p q c", q=GRP),
                                axis=AX.X, op=OP.min,
                            )

            # ---------- Phase D: per-node algebra ----------
            with tc.tile_pool(name="psD", bufs=1, space="PSUM") as psD:
                if _VARIANT == "optA":
                    # sumc holds sum(dist + nsx) = sum - 512*sx ; undo the shift
                    t512 = pers.tile([128, NT], f32)
                    nc.vector.tensor_scalar(t512[:], nsx[:], -512.0, None, op0=OP.mult)
                    nc.vector.tensor_add(sumc[:], sumc[:], t512[:])
                    # hard = min16 - nsx
                    nc.vector.tensor_sub(minc[:], min16[:], nsx[:])
                own2 = pers.tile([128, NT], f32)
                nc.vector.tensor_mul(own2[:], ownc[:], ownc[:])
                tx = pers.tile([128, NT], f32)
                nc.vector.tensor_add(tx[:], x2c[:], c2cols[:])
                nc.vector.tensor_sub(tx[:], tx[:], own2[:])
                xS = pers.tile([128, NT], f32)
                nc.vector.tensor_mul(xS[:], tx[:], cntc[:])
                nc.vector.tensor_scalar(xS[:], xS[:], 0.5, None, op0=OP.mult)
                S2 = pers.tile([128, NT], f32)
                nc.vector.tensor_mul(S2[:], c2cols[:], cnt2c[:])
                ap1 = pers.tile([128, NT], f32)
                nc.vector.tensor_scalar(ap1[:], acol[:], 1.0, None, op0=OP.add)
                aa = pers.tile([128, NT], f32)
                nc.vector.tensor_mul(aa[:], acol[:], acol[:])
                pos2 = pers.tile([128, NT], f32)
                nc.vector.tensor_mul(pos2[:], aa[:], S2[:])
                t2 = pers.tile([128, NT], f32)
                nc.vector.tensor_mul(t2[:], ap1[:], ap1[:])
                nc.vector.tensor_mul(t2[:], t2[:], x2c[:])
                nc.vector.tensor_add(pos2[:], pos2[:], t2[:])
                t3 = pers.tile([128, NT], f32)
                nc.vector.tensor_mul(t3[:], acol[:], ap1[:])
                nc.vector.tensor_mul(t3[:], t3[:], xS[:])
                nc.vector.tensor_scalar(t3[:], t3[:], -2.0, None, op0=OP.mult)
                nc.vector.tensor_add(pos2[:], pos2[:], t3[:])
                nc.vector.tensor_scalar(pos2[:], pos2[:], 0.0, None, op0=OP.max)
                pos = pers.tile([128, NT], f32)
                nc.scalar.activation(pos[:], pos2[:], ACTF.Sqrt)
                mneg = pers.tile([128, NT], f32)
                nc.vector.tensor_sub(mneg[:], sumc[:], ownc[:])
                nc.vector.tensor_scalar(mneg[:], mneg[:], 1.0 / (C - 1), None, op0=OP.mult)
                tm = pers.tile([128, NT], f32)
                nc.vector.tensor_sub(tm[:], pos[:], mneg[:])
                nc.vector.tensor_scalar(tm[:], tm[:], ALPHA, 0.0, op0=OP.add, op1=OP.max)
                nc.vector.tensor_mul(tm[:], tm[:], valid[:])
                tn = pers.tile([128, NT], f32)
                nc.vector.tensor_sub(tn[:], pos[:], minc[:])
                nc.vector.tensor_scalar(tn[:], tn[:], ALPHA, 0.0, op0=OP.add, op1=OP.max)
                nc.vector.tensor_mul(tn[:], tn[:], valid[:])
                sq = pers.tile([128, CPC], f32)
                nc.vector.tensor_mul(sq[:], msq[:, 0:CPC], cntbc[:])
                nc.vector.tensor_scalar(sq[:], sq[:], -0.25, None, op0=OP.mult)
                nc.vector.tensor_add(sq[:], sq[:], SQloc[:])
                nc.vector.tensor_mul(sq[:], sq[:], cm1bc[:])
                nc.vector.tensor_scalar(sq[:], sq[:], 0.0, None, op0=OP.max)
                stdv = pers.tile([128, CPC], f32)
                nc.scalar.activation(stdv[:], sq[:], ACTF.Sqrt)
                nc.vector.tensor_scalar(stdv[:], stdv[:], -1.0, None, op0=OP.add)
                nc.vector.tensor_mul(stdv[:], stdv[:], stdv[:])
                red = pers.tile([128, 4], f32)
                nc.vector.tensor_reduce(red[:, 0:1], tm[:], axis=AX.X, op=OP.add)
                nc.vector.tensor_reduce(red[:, 1:2], tn[:], axis=AX.X, op=OP.add)
                nc.vector.tensor_reduce(red[:, 2:3], stdv[:], axis=AX.X, op=OP.add)
                nc.vector.memset(red[:, 3:4], 0.0)
                ps_f = psD.tile([4, 1], f32, tag="psf")
                nc.tensor.matmul(ps_f[:], red[:], onescol[:], start=True, stop=True)
                outsb = pers.tile([4, 1], f32)
                nc.vector.tensor_copy(outsb[:], ps_f[:])
                nc.gpsimd.dma_start(out_d[:], outsb[:])
                dbg = pers.tile([128, 8], f32)
                nc.vector.tensor_copy(dbg[:, 0:1], sumc[:, 0:1])
                nc.vector.tensor_copy(dbg[:, 1:2], ownc[:, 0:1])
                nc.vector.tensor_copy(dbg[:, 2:3], minc[:, 0:1])
                nc.vector.tensor_copy(dbg[:, 3:4], pos[:, 0:1])
                nc.vector.tensor_copy(dbg[:, 4:5], mneg[:, 0:1])
                nc.vector.tensor_copy(dbg[:, 5:6], x2c[:, 0:1])
                nc.vector.tensor_copy(dbg[:, 6:7], Srot[:, 0:1])
                nc.vector.tensor_copy(dbg[:, 7:8], c2cols[:, 0:1])
                nc.gpsimd.dma_start(dbg_d[:], dbg[:])

    nc.compile()
    return nc


def _host_prep(node_features, community_belong_list):
    x = np.asarray(node_features)
    if x.ndim == 3:
        x = x[0]
    x = np.ascontiguousarray(x, dtype=np.float32)  # [N, D]
    cbl = np.asarray(community_belong_list).astype(np.int64)
    N = x.shape[0]
    comm = np.empty(N, np.int64)
    comm[cbl[0]] = cbl[1]
    counts = np.bincount(comm, minlength=C).astype(np.int64)
    assert counts.min() >= 2, f"community with <2 nodes: {counts.min()}"
    assert counts.max() <= KSLOT, f"community overflow: {counts.max()} > {KSLOT}"

    order = np.argsort(comm, kind="stable")
    comm_sorted = comm[order]
    cstart = np.zeros(C + 1, np.int64)
    np.cumsum(counts, out=cstart[1:])
    ranks = np.arange(N) - cstart[comm_sorted]
    slot_global = comm_sorted * KSLOT + ranks

    X_pad = np.zeros((C * KSLOT, D), np.float32)
    X_pad[slot_global] = x[order]
    x2 = (x.astype(np.float64) ** 2).sum(1).astype(np.float32)
    x2_pad = np.zeros(C * KSLOT, np.float32)
    x2_pad[slot_global] = x2[order]
    valid_pad = np.zeros(C * KSLOT, np.float32)
    valid_pad[slot_global] = 1.0

    countsf = counts.astype(np.float32)
    ident = np.eye(128, dtype=np.float32)

    in_maps = []
    for k in range(NCORES):
        lo = k * SLOTS
        hi = lo + SLOTS
        xTk = np.ascontiguousarray(X_pad[lo:hi].T).astype(ml_dtypes.bfloat16)
        x2k = np.ascontiguousarray(x2_pad[lo:hi].reshape(NT, 128).T)
        vk = np.ascontiguousarray(valid_pad[lo:hi].reshape(NT, 128).T)
        ccore = counts[k * CPC : (k + 1) * CPC].astype(np.float32)
        cnt_col = np.repeat(ccore, 2)
        cntc = np.broadcast_to(cnt_col, (128, NT)).copy()
        acol = np.broadcast_to(1.0 / (cnt_col - 1.0), (128, NT)).astype(np.float32).copy()
        cnt2c = np.broadcast_to(cnt_col * cnt_col, (128, NT)).astype(np.float32).copy()
        ntcr = np.roll(-2.0 / countsf, -k * CPC).reshape(1, C).astype(np.float32)
        cntrow = ccore.reshape(1, CPC)
        cm1r = (1.0 / (ccore - 1.0)).reshape(1, CPC).astype(np.float32)
        pplace = np.zeros((CPC, C), np.float32)
        pplace[np.arange(CPC), k * CPC + np.arange(CPC)] = 1.0
        prot = np.zeros((C, C), np.float32)
        src = (k * CPC + np.arange(C)) % C
        prot[src, np.arange(C)] = 1.0
        m = {
            "xT": xTk,
            "x2c": x2k,
            "valid": vk,
            "ntcr": ntcr,
            "cntc": cntc.astype(np.float32),
            "acol": acol,
            "cnt2c": cnt2c,
            "cntrow": cntrow.astype(np.float32),
            "cm1r": cm1r,
            "pplace": pplace,
            "prot": prot,
            "ident": ident,
        }
        if _VARIANT == "optA":
            x2slot = x2_pad[lo:hi]
            x2hi = x2slot.astype(ml_dtypes.bfloat16)
            x2lo = (x2slot - x2hi.astype(np.float32)).astype(ml_dtypes.bfloat16)
            m["x2r"] = np.ascontiguousarray(np.stack([x2hi, x2lo]))  # [2, SLOTS] bf16
            m["nsx"] = np.ascontiguousarray(
                (-np.sqrt(x2_pad[lo:hi])).reshape(NT, 128).T
            ).astype(np.float32)
        in_maps.append(m)
    return in_maps


def kernel(node_features, community_belong_list):
    global _PROG
    in_maps = _host_prep(node_features, community_belong_list)
    if _PROG is None:
        _PROG = _build_program()
    r = run_bass_kernel_spmd(_PROG, in_maps, list(range(NCORES)))
    tm = sum(float(r.results[k]["out"][0, 0]) for k in range(NCORES))
    tn = sum(float(r.results[k]["out"][1, 0]) for k in range(NCORES))
    sd = sum(float(r.results[k]["out"][2, 0]) for k in range(NCORES))
    kernel._last_results = r
    out = np.array([tm / N_NODES, tn / N_NODES, sd / (C * D)], dtype=np.float32)
    return out



# revision 14
# speedup vs baseline: 1.6608x; 1.6608x over previous
"""Trainium2 Bass kernel for nn_CDTripletLoss (segment_reduce).

Strategy: community-sharded data layout. Host sorts nodes by community,
pads each community to 256 slots, assigns 64 communities per core.
Device computes per-community stats (bn_stats), one bf16 AllGather of the
[64,128] local community sums, then per-node distances via bf16 matmuls
([node,comm] tiles), sqrt on ScalarE writing fp16 directly, row-sum on
VectorE, masked min split between VectorE and the Pool engine, and the
triplet/std losses via per-node column algebra. Host combines the 8
cores' partial sums.
"""
import numpy as np
import ml_dtypes

import concourse.bass as bass
import concourse.tile as tile
from concourse import bacc, mybir
from concourse.bass_utils import run_bass_kernel_spmd

f32 = mybir.dt.float32
bf16 = mybir.dt.bfloat16
fp16 = mybir.dt.float16
AX = mybir.AxisListType
OP = mybir.AluOpType
ACTF = mybir.ActivationFunctionType

NCORES = 8
C = 512            # communities
CPC = 64           # communities per core
KSLOT = 256        # padded slots per community
SLOTS = CPC * KSLOT   # 16384 slots per core
D = 128
NT = SLOTS // 128  # 128 node tiles per core
GRP = 4            # tiles per dist group
NGRP = NT // GRP
CHUNK = 2048       # slots per load chunk
NCH = SLOTS // CHUNK
N_NODES = 100000
ALPHA = 0.25
BIG16 = 60000.0

_PROG = None
_NO_CC = False     # timing-only: replace the collective with a local DMA


def _build_program():
    nc = bacc.Bacc("TRN2", target_bir_lowering=False, debug=False, num_devices=NCORES)

    xT_in = nc.declare_dram_parameter("xT", [D, SLOTS], bf16, isOutput=False)
    aug2_in = nc.declare_dram_parameter("aug2", [2, SLOTS], bf16, isOutput=False)
    x2c_in = nc.declare_dram_parameter("x2c", [128, NT], f32, isOutput=False)
    valid_in = nc.declare_dram_parameter("valid", [128, NT], f32, isOutput=False)
    cntc_in = nc.declare_dram_parameter("cntc", [128, NT], f32, isOutput=False)
    acol_in = nc.declare_dram_parameter("acol", [128, NT], f32, isOutput=False)
    cnt2c_in = nc.declare_dram_parameter("cnt2c", [128, NT], f32, isOutput=False)
    cntrow_in = nc.declare_dram_parameter("cntrow", [1, CPC], f32, isOutput=False)
    cm1r_in = nc.declare_dram_parameter("cm1r", [1, CPC], f32, isOutput=False)
    prot_in = nc.declare_dram_parameter("prot", [C, C], bf16, isOutput=False)
    ident_in = nc.declare_dram_parameter("ident", [128, 128], bf16, isOutput=False)

    out_d = nc.declare_dram_parameter("out", [4, 1], f32, isOutput=True)

    with tile.TileContext(nc, num_cores=NCORES) as tc:
        with (
            tc.tile_pool(name="pers", bufs=1) as pers,
            tc.tile_pool(name="dist", bufs=3) as distp,
            tc.tile_pool(name="dram", bufs=1, space="DRAM") as dramp,
        ):
            # ---------- Phase 0: loads (HWDGE: SP for xT, ACT for aux) ----------
            xTc = []
            for ch in range(NCH):
                t = pers.tile([D, CHUNK], bf16, tag=f"xt{ch}")
                nc.sync.dma_start(t[:], xT_in[:, CHUNK * ch : CHUNK * (ch + 1)])
                xTc.append(t)
            aug2 = pers.tile([2, SLOTS], bf16)
            nc.scalar.dma_start(aug2[:], aug2_in[:])
            cntrow = pers.tile([1, CPC], f32)
            nc.scalar.dma_start(cntrow[:], cntrow_in[:])
            cm1r = pers.tile([1, CPC], f32)
            nc.scalar.dma_start(cm1r[:], cm1r_in[:])
            ident = pers.tile([128, 128], bf16)
            nc.scalar.dma_start(ident[:], ident_in[:])
            x2c = pers.tile([128, NT], f32)
            nc.scalar.dma_start(x2c[:], x2c_in[:])
            acol = pers.tile([128, NT], f32)
            nc.scalar.dma_start(acol[:], acol_in[:])
            prot = pers.tile([128, 4, C], bf16)
            nc.scalar.dma_start(prot[:], prot_in.rearrange("(ch c) f -> c ch f", ch=4))
            valid = pers.tile([128, NT], f32)
            nc.scalar.dma_start(valid[:], valid_in[:])
            cntc = pers.tile([128, NT], f32)
            nc.scalar.dma_start(cntc[:], cntc_in[:])
            cnt2c = pers.tile([128, NT], f32)
            nc.scalar.dma_start(cnt2c[:], cnt2c_in[:])

            ones1 = pers.tile([1, 128], f32)
            nc.vector.memset(ones1[:], 1.0)
            ones1b = pers.tile([1, 128], bf16)
            nc.vector.memset(ones1b[:], 1.0)
            onescol = pers.tile([128, 1], f32)
            nc.vector.memset(onescol[:], 1.0)
            col025 = pers.tile([128, 1], bf16)
            nc.vector.memset(col025[:], 0.25)
            rhs2 = pers.tile([2, C], bf16)
            nc.vector.memset(rhs2[:], 1.0)

            # ---------- Phase A: per-community stats ----------
            # bn_stats over [128, 256] -> [128, 6]: (cnt, mean, M2) per 128-half
            bnb = pers.tile([128, CPC, 6], f32)
            CPCH = CHUNK // KSLOT  # communities per chunk
            for ch in range(NCH):
                for p in range(CPCH):
                    j = CPCH * ch + p
                    nc.vector.bn_stats(bnb[:, j, :],
                                       xTc[ch][:, KSLOT * p : KSLOT * (p + 1)])
            mA = bnb[:, :, 1]
            m2A = bnb[:, :, 2]
            mB = bnb[:, :, 4]
            m2B = bnb[:, :, 5]
            Sloc = pers.tile([128, CPC], f32)
            nc.vector.tensor_add(Sloc[:], mA, mB)
            nc.vector.tensor_scalar(Sloc[:], Sloc[:], 128.0, None, op0=OP.mult)
            SQloc = pers.tile([128, CPC], f32)
            tA = pers.tile([128, CPC], f32)
            nc.vector.tensor_mul(tA[:], mA, mA)
            tB = pers.tile([128, CPC], f32)
            nc.vector.tensor_mul(tB[:], mB, mB)
            nc.vector.tensor_add(tA[:], tA[:], tB[:])
            nc.vector.tensor_scalar(tA[:], tA[:], 128.0, None, op0=OP.mult)
            nc.vector.tensor_add(SQloc[:], m2A, m2B)
            nc.vector.tensor_add(SQloc[:], SQloc[:], tA[:])

            # ---------- Phase B: collective + means prep ----------
            with tc.tile_pool(name="psB", bufs=2, space="PSUM") as psB:
                Sb16 = pers.tile([128, CPC], bf16)
                nc.vector.tensor_copy(Sb16[:], Sloc[:])
                ps_t = psB.tile([CPC, 128], bf16, tag="ps")
                nc.tensor.transpose(ps_t[:], Sb16[:], ident[:])
                S_jd = pers.tile([CPC, 128], bf16)
                nc.vector.tensor_copy(S_jd[:], ps_t[:])
                ccin = dramp.tile([CPC, D], bf16)
                nc.sync.dma_start(ccin[:], S_jd[:])
                ccout = dramp.tile([C, D], bf16)
                if _NO_CC:
                    for r in range(NCORES):
                        nc.sync.dma_start(ccout[CPC * r : CPC * (r + 1), :], ccin[:])
                else:
                    nc.gpsimd.collective_compute(
                        "AllGather", OP.bypass,
                        replica_groups=[list(range(NCORES))],
                        ins=[ccin[:].opt()],
                        outs=[ccout[:].opt()],
                    )
                gsb = pers.tile([128, 4, 128], bf16)
                nc.sync.dma_start(gsb[:], ccout.rearrange("(ch c) d -> c ch d", ch=4))

                # independent precompute while the collective runs
                ps_cb = psB.tile([128, CPC], f32, tag="ps")
                nc.tensor.matmul(ps_cb[:], ones1[:], cntrow[:], start=True, stop=True)
                cntbc = pers.tile([128, CPC], f32)
                nc.vector.tensor_copy(cntbc[:], ps_cb[:])
                ps_cb2 = psB.tile([128, CPC], f32, tag="ps")
                nc.tensor.matmul(ps_cb2[:], ones1[:], cm1r[:], start=True, stop=True)
                cm1bc = pers.tile([128, CPC], f32)
                nc.vector.tensor_copy(cm1bc[:], ps_cb2[:])
                ap1 = pers.tile([128, NT], f32)
                nc.vector.tensor_scalar(ap1[:], acol[:], 1.0, None, op0=OP.add)
                aa = pers.tile([128, NT], f32)
                nc.vector.tensor_mul(aa[:], acol[:], acol[:])
                t2 = pers.tile([128, NT], f32)
                nc.vector.tensor_mul(t2[:], ap1[:], ap1[:])
                nc.vector.tensor_mul(t2[:], t2[:], x2c[:])

                # rotated means: m2T = -2*mean, scale folded into prot values
                ps_r = psB.tile([128, C], f32, tag="ps")
                for ch in range(4):
                    nc.tensor.matmul(ps_r[:], gsb[:, ch, :], prot[:, ch, :],
                                     start=(ch == 0), stop=(ch == 3))
                m2T = pers.tile([128, C], bf16)
                nc.scalar.copy(m2T[:], ps_r[:])
                msqb = pers.tile([128, C], bf16)
                nc.vector.tensor_mul(msqb[:], m2T[:], m2T[:])
                ps_c = psB.tile([1, C], f32, tag="ps")
                nc.tensor.matmul(ps_c[:], col025[:], msqb[:], start=True, stop=True)
                c2f = pers.tile([1, C], f32)
                nc.vector.tensor_copy(c2f[:], ps_c[:])
                nc.vector.tensor_copy(rhs2[0:1, :], c2f[:])
                c2rep = pers.tile([1, NT], bf16)
                nc.vector.tensor_copy(c2rep[0:1, 0:NT:2], c2f[0:1, 0:CPC])
                nc.vector.tensor_copy(c2rep[0:1, 1:NT:2], c2f[0:1, 0:CPC])
                ps_cc = psB.tile([128, NT], f32, tag="ps")
                nc.tensor.matmul(ps_cc[:], ones1b[:], c2rep[:], start=True, stop=True)
                c2cols = pers.tile([128, NT], f32)
                nc.vector.tensor_copy(c2cols[:], ps_cc[:])

            # ---------- Phase C: distance tiles ----------
            sumc = pers.tile([128, NT], f32)
            ownc = pers.tile([128, NT], f32)
            min16 = pers.tile([128, NT], fp16)
            MINW = 64  # per-tile min tree stops at this width
            TPCH = CHUNK // 128  # node tiles per chunk
            with tc.tile_pool(name="psC", bufs=2, space="PSUM") as psC:
                for g in range(NGRP):
                    d16 = distp.tile([128, GRP * C], fp16)
                    ps4 = psC.tile([128, GRP * C], f32, tag="ps")
                    for q in range(GRP):
                        t = GRP * g + q
                        nc.tensor.matmul(ps4[:, C * q : C * (q + 1)],
                                         xTc[t // TPCH][:, 128 * (t % TPCH) : 128 * (t % TPCH + 1)],
                                         m2T[:], start=True, stop=False)
                        nc.tensor.matmul(ps4[:, C * q : C * (q + 1)],
                                         aug2[:, 128 * t : 128 * (t + 1)],
                                         rhs2[:], start=False, stop=True)
                    nc.scalar.activation(d16[:], ps4[:], ACTF.Sqrt)
                    for q in range(GRP):
                        t = GRP * g + q
                        nc.vector.tensor_scalar(
                            d16[:, C * q : C * (q + 1)], d16[:, C * q : C * (q + 1)],
                            0.0, 0.0, op0=OP.add, op1=OP.add,
                            accum_out=sumc[:, t : t + 1])
                    # own cols of tiles 4g..4g+3 sit at stride-C pairs:
                    # [2g, C+2g] and [2C+2g+1, 3C+2g+1]
                    o0 = 2 * g
                    o1 = 2 * C + 2 * g + 1
                    nc.gpsimd.tensor_copy(
                        ownc[:, 4 * g : 4 * g + 2],
                        d16[:, o0 : o0 + C + 1 : C])
                    nc.gpsimd.tensor_copy(
                        ownc[:, 4 * g + 2 : 4 * g + 4],
                        d16[:, o1 : o1 + C + 1 : C])
                    nc.gpsimd.memset(d16[:, o0 : o0 + C + 1 : C], BIG16)
                    nc.gpsimd.memset(d16[:, o1 : o1 + C + 1 : C], BIG16)
                    # binary TT-min tree per tile, in place: width 512 -> MINW
                    w = C
                    v = d16.rearrange("p (q c) -> p q c", q=GRP)
                    while w > MINW:
                        h = w // 2
                        nc.vector.tensor_tensor(
                            v[:, :, 0:h], v[:, :, 0:h], v[:, :, h:w], op=OP.min)
                        w = h
                    nc.vector.tensor_reduce(
                        min16[:, GRP * g : GRP * (g + 1)],
                        v[:, :, 0:MINW], axis=AX.X, op=OP.min)

            # ---------- Phase D: per-node algebra ----------
            with tc.tile_pool(name="psD", bufs=1, space="PSUM") as psD:
                minc = pers.tile([128, NT], f32)
                nc.vector.tensor_copy(minc[:], min16[:])
                own2 = pers.tile([128, NT], f32)
                nc.vector.tensor_mul(own2[:], ownc[:], ownc[:])
                tx = pers.tile([128, NT], f32)
                nc.vector.tensor_add(tx[:], x2c[:], c2cols[:])
                nc.vector.tensor_sub(tx[:], tx[:], own2[:])
                xS = pers.tile([128, NT], f32)
                nc.vector.tensor_mul(xS[:], tx[:], cntc[:])
                nc.vector.tensor_scalar(xS[:], xS[:], 0.5, None, op0=OP.mult)
                S2 = pers.tile([128, NT], f32)
                nc.vector.tensor_mul(S2[:], c2cols[:], cnt2c[:])
                pos2 = pers.tile([128, NT], f32)
                nc.vector.tensor_mul(pos2[:], aa[:], S2[:])
                nc.vector.tensor_add(pos2[:], pos2[:], t2[:])
                t3 = pers.tile([128, NT], f32)
                nc.vector.tensor_mul(t3[:], acol[:], ap1[:])
                nc.vector.tensor_mul(t3[:], t3[:], xS[:])
                nc.vector.tensor_scalar(t3[:], t3[:], -2.0, None, op0=OP.mult)
                nc.vector.tensor_add(pos2[:], pos2[:], t3[:])
                nc.vector.tensor_scalar(pos2[:], pos2[:], 0.0, None, op0=OP.max)
                pos = pers.tile([128, NT], f32)
                nc.scalar.activation(pos[:], pos2[:], ACTF.Sqrt)
                mneg = pers.tile([128, NT], f32)
                nc.vector.tensor_sub(mneg[:], sumc[:], ownc[:])
                nc.vector.tensor_scalar(mneg[:], mneg[:], 1.0 / (C - 1), None, op0=OP.mult)
                tm = pers.tile([128, NT], f32)
                nc.vector.tensor_sub(tm[:], pos[:], mneg[:])
                nc.vector.tensor_scalar(tm[:], tm[:], ALPHA, 0.0, op0=OP.add, op1=OP.max)
                nc.vector.tensor_mul(tm[:], tm[:], valid[:])
                tn = pers.tile([128, NT], f32)
                nc.vector.tensor_sub(tn[:], pos[:], minc[:])
                nc.vector.tensor_scalar(tn[:], tn[:], ALPHA, 0.0, op0=OP.add, op1=OP.max)
                nc.vector.tensor_mul(tn[:], tn[:], valid[:])
                sq = pers.tile([128, CPC], f32)
                nc.vector.tensor_mul(sq[:], msqb[:, 0:CPC], cntbc[:])
                nc.vector.tensor_scalar(sq[:], sq[:], -0.25, None, op0=OP.mult)
                nc.vector.tensor_add(sq[:], sq[:], SQloc[:])
                nc.vector.tensor_mul(sq[:], sq[:], cm1bc[:])
                nc.vector.tensor_scalar(sq[:], sq[:], 0.0, None, op0=OP.max)
                stdv = pers.tile([128, CPC], f32)
                nc.scalar.activation(stdv[:], sq[:], ACTF.Sqrt)
                nc.vector.tensor_scalar(stdv[:], stdv[:], -1.0, None, op0=OP.add)
                nc.vector.tensor_mul(stdv[:], stdv[:], stdv[:])
                red = pers.tile([128, 4], f32)
                nc.vector.tensor_reduce(red[:, 0:1], tm[:], axis=AX.X, op=OP.add)
                nc.vector.tensor_reduce(red[:, 1:2], tn[:], axis=AX.X, op=OP.add)
                nc.vector.tensor_reduce(red[:, 2:3], stdv[:], axis=AX.X, op=OP.add)
                nc.vector.memset(red[:, 3:4], 0.0)
                ps_f = psD.tile([4, 1], f32, tag="psf")
                nc.tensor.matmul(ps_f[:], red[:], onescol[:], start=True, stop=True)
                outsb = pers.tile([4, 1], f32)
                nc.vector.tensor_copy(outsb[:], ps_f[:])
                nc.sync.dma_start(out_d[:], outsb[:])

    nc.compile()
    return nc


def _host_prep(node_features, community_belong_list):
    x = np.asarray(node_features)
    if x.ndim == 3:
        x = x[0]
    x = np.ascontiguousarray(x, dtype=np.float32)  # [N, D]
    cbl = np.asarray(community_belong_list).astype(np.int64)
    N = x.shape[0]
    comm = np.empty(N, np.int64)
    comm[cbl[0]] = cbl[1]
    counts = np.bincount(comm, minlength=C).astype(np.int64)
    assert counts.min() >= 2, f"community with <2 nodes: {counts.min()}"
    assert counts.max() <= KSLOT, f"community overflow: {counts.max()} > {KSLOT}"

    order = np.argsort(comm, kind="stable")
    comm_sorted = comm[order]
    cstart = np.zeros(C + 1, np.int64)
    np.cumsum(counts, out=cstart[1:])
    ranks = np.arange(N) - cstart[comm_sorted]
    slot_global = comm_sorted * KSLOT + ranks

    X_pad = np.zeros((C * KSLOT, D), np.float32)
    X_pad[slot_global] = x[order]
    x2 = (x.astype(np.float64) ** 2).sum(1).astype(np.float32)
    x2_pad = np.zeros(C * KSLOT, np.float32)
    x2_pad[slot_global] = x2[order]
    valid_pad = np.zeros(C * KSLOT, np.float32)
    valid_pad[slot_global] = 1.0

    countsf = counts.astype(np.float32)
    ident = np.eye(128, dtype=ml_dtypes.bfloat16)

    in_maps = []
    for k in range(NCORES):
        lo = k * SLOTS
        hi = lo + SLOTS
        xTk = np.ascontiguousarray(X_pad[lo:hi].T).astype(ml_dtypes.bfloat16)
        x2k = np.ascontiguousarray(x2_pad[lo:hi].reshape(NT, 128).T)
        vk = np.ascontiguousarray(valid_pad[lo:hi].reshape(NT, 128).T)
        ccore = counts[k * CPC : (k + 1) * CPC].astype(np.float32)
        cnt_col = np.repeat(ccore, 2)
        cntc = np.broadcast_to(cnt_col, (128, NT)).copy()
        acol = np.broadcast_to(1.0 / (cnt_col - 1.0), (128, NT)).astype(np.float32).copy()
        cnt2c = np.broadcast_to(cnt_col * cnt_col, (128, NT)).astype(np.float32).copy()
        cntrow = ccore.reshape(1, CPC)
        cm1r = (1.0 / (ccore - 1.0)).reshape(1, CPC).astype(np.float32)
        src = (k * CPC + np.arange(C)) % C
        prot = np.zeros((C, C), np.float32)
        prot[src, np.arange(C)] = -2.0 / countsf[src]
        aug2 = np.ones((2, SLOTS), np.float32)
        aug2[1] = x2_pad[lo:hi]
        m = {
            "xT": xTk,
            "aug2": aug2.astype(ml_dtypes.bfloat16),
            "x2c": x2k,
            "valid": vk,
            "cntc": cntc.astype(np.float32),
            "acol": acol,
            "cnt2c": cnt2c,
            "cntrow": cntrow.astype(np.float32),
            "cm1r": cm1r,
            "prot": prot.astype(ml_dtypes.bfloat16),
            "ident": ident,
        }
        in_maps.append(m)
    return in_maps


def kernel(node_features, community_belong_list):
    global _PROG
    in_maps = _host_prep(node_features, community_belong_list)
    if _PROG is None:
        _PROG = _build_program()
    r = run_bass_kernel_spmd(_PROG, in_maps, list(range(NCORES)))
    tm = sum(float(r.results[k]["out"][0, 0]) for k in range(NCORES))
    tn = sum(float(r.results[k]["out"][1, 0]) for k in range(NCORES))
    sd = sum(float(r.results[k]["out"][2, 0]) for k in range(NCORES))
    kernel._last_results = r
    out = np.array([tm / N_NODES, tn / N_NODES, sd / (C * D)], dtype=np.float32)
    return out


# revision 34
# speedup vs baseline: 1.8049x; 1.0868x over previous
"""Trainium2 Bass kernel for nn_CDTripletLoss (segment_reduce).

Strategy: community-sharded data layout. Host sorts nodes by community,
pads each community to 256 slots, assigns 64 communities per core.
Device computes per-community stats (bn_stats), one bf16 AllGather of the
[64,128] local community sums, then per-node distances via bf16 matmuls
([node,comm] tiles), sqrt on ScalarE writing fp16 directly, column sums +
an in-place tensor-tensor min tree on VectorE, own-column handling on the
Pool engine, and the triplet/std losses via per-node column algebra with
host-folded constants. Host combines the 8 cores' partial sums.
"""
import numpy as np
import ml_dtypes

import concourse.bass as bass
import concourse.tile as tile
from concourse import bacc, mybir
from concourse.bass_utils import run_bass_kernel_spmd

f32 = mybir.dt.float32
bf16 = mybir.dt.bfloat16
fp16 = mybir.dt.float16
fp8 = mybir.dt.float8e4
AX = mybir.AxisListType
OP = mybir.AluOpType
ACTF = mybir.ActivationFunctionType

NCORES = 8
C = 512            # communities
CPC = 64           # communities per core
KSLOT = 256        # padded slots per community
SLOTS = CPC * KSLOT   # 16384 slots per core
D = 128
NT = SLOTS // 128  # 128 node tiles per core
GRP = 4            # tiles per dist group
NGRP = NT // GRP
CHUNK = 2048       # slots per load chunk
NCH = SLOTS // CHUNK
N_NODES = 100000
ALPHA = 0.25
BIG16 = 60000.0
NEGBIG = -1.0e6

_PROG = None
_NO_CC = False     # timing-only: replace the collective with a local DMA


def _build_program():
    nc = bacc.Bacc("TRN2", target_bir_lowering=False, debug=False, num_devices=NCORES)

    xT_in = nc.declare_dram_parameter("xT", [D, SLOTS], bf16, isOutput=False)
    aug2_in = nc.declare_dram_parameter("aug2", [2, SLOTS], bf16, isOutput=False)
    x2c_in = nc.declare_dram_parameter("x2c", [128, NT], f32, isOutput=False)
    ca2_in = nc.declare_dram_parameter("ca2", [128, NT], f32, isOutput=False)
    cnt2c_in = nc.declare_dram_parameter("cnt2c", [128, NT], f32, isOutput=False)
    aa_in = nc.declare_dram_parameter("aa", [128, NT], f32, isOutput=False)
    t2_in = nc.declare_dram_parameter("t2", [128, NT], f32, isOutput=False)
    cntrow_in = nc.declare_dram_parameter("cntrow", [1, CPC], f32, isOutput=False)
    cm1r_in = nc.declare_dram_parameter("cm1r", [1, CPC], f32, isOutput=False)
    prot_in = nc.declare_dram_parameter("prot", [C, C], fp8, isOutput=False)
    ident_in = nc.declare_dram_parameter("ident", [128, 128], bf16, isOutput=False)

    out_d = nc.declare_dram_parameter("out", [128, 4], f32, isOutput=True)

    with tile.TileContext(nc, num_cores=NCORES) as tc:
        with (
            tc.tile_pool(name="pers", bufs=1) as pers,
            tc.tile_pool(name="dist", bufs=4) as distp,
            tc.tile_pool(name="dram", bufs=1, space="DRAM") as dramp,
        ):
            # ---------- Phase 0: loads (HWDGE: SP for xT, ACT for aux) ----------
            # small leading chunks let bn_stats start as early as possible
            sizes = [512, 512, 1024] + [2048] * 7
            assert sum(sizes) == SLOTS
            xTc = []
            off = 0
            for ci, sz in enumerate(sizes):
                t = pers.tile([D, sz], bf16, tag=f"xt{ci}")
                nc.sync.dma_start(t[:], xT_in[:, off : off + sz])
                xTc.append((t, off, sz))
                off += sz

            def xsl(tidx):
                base = 128 * tidx
                for t, o, sz in xTc:
                    if o <= base < o + sz:
                        return t[:, base - o : base - o + 128]
                raise AssertionError(tidx)
            aug2 = pers.tile([2, SLOTS], bf16)
            nc.scalar.dma_start(aug2[:], aug2_in[:])
            cntrow = pers.tile([1, CPC], f32)
            nc.scalar.dma_start(cntrow[:], cntrow_in[:])
            cm1r = pers.tile([1, CPC], f32)
            nc.scalar.dma_start(cm1r[:], cm1r_in[:])
            ident = pers.tile([128, 128], bf16)
            nc.scalar.dma_start(ident[:], ident_in[:])
            x2c = pers.tile([128, NT], f32)
            nc.scalar.dma_start(x2c[:], x2c_in[:])
            prot = pers.tile([128, 4, C], fp8)
            nc.scalar.dma_start(prot[:], prot_in.rearrange("(ch c) f -> c ch f", ch=4))
            ca2 = pers.tile([128, NT], f32)
            nc.scalar.dma_start(ca2[:], ca2_in[:])
            cnt2c = pers.tile([128, NT], f32)
            nc.scalar.dma_start(cnt2c[:], cnt2c_in[:])
            aa = pers.tile([128, NT], f32)
            nc.scalar.dma_start(aa[:], aa_in[:])
            t2 = pers.tile([128, NT], f32)
            nc.scalar.dma_start(t2[:], t2_in[:])

            ones1 = pers.tile([1, 128], f32)
            nc.gpsimd.memset(ones1[:], 1.0)
            ones1b = pers.tile([1, 128], bf16)
            nc.gpsimd.memset(ones1b[:], 1.0)
            col025 = pers.tile([128, 1], bf16)
            nc.gpsimd.memset(col025[:], 0.25)
            rhs2 = pers.tile([2, C], bf16)
            nc.gpsimd.memset(rhs2[:], 1.0)
            # preload the Sqrt table so phase B/C activations start instantly
            junk8 = pers.tile([1, 8], f32)
            nc.scalar.activation(junk8[:], ones1[0:1, 0:8], ACTF.Sqrt)

            # ---------- Phase A: per-community stats ----------
            # bn_stats over [128, 256] -> [128, 6]: (cnt, mean, M2) per 128-half
            bnb = pers.tile([128, CPC, 6], f32)
            for t, o, sz in xTc:
                for p in range(sz // KSLOT):
                    j = o // KSLOT + p
                    nc.vector.bn_stats(bnb[:, j, :],
                                       t[:, KSLOT * p : KSLOT * (p + 1)])
            mA = bnb[:, :, 1]
            m2A = bnb[:, :, 2]
            mB = bnb[:, :, 4]
            m2B = bnb[:, :, 5]
            Sloc = pers.tile([128, CPC], f32)
            nc.vector.tensor_add(Sloc[:], mA, mB)
            nc.vector.tensor_scalar(Sloc[:], Sloc[:], 128.0, None, op0=OP.mult)
            SQloc = pers.tile([128, CPC], f32)
            tA = pers.tile([128, CPC], f32)
            nc.vector.tensor_mul(tA[:], mA, mA)
            tB = pers.tile([128, CPC], f32)
            nc.vector.tensor_mul(tB[:], mB, mB)
            nc.vector.tensor_add(tA[:], tA[:], tB[:])
            nc.vector.tensor_scalar(tA[:], tA[:], 128.0, None, op0=OP.mult)
            nc.vector.tensor_add(SQloc[:], m2A, m2B)
            nc.vector.tensor_add(SQloc[:], SQloc[:], tA[:])

            # ---------- Phase B: collective + means prep ----------
            with tc.tile_pool(name="psB", bufs=2, space="PSUM") as psB:
                Sb16 = pers.tile([128, CPC], bf16)
                nc.vector.tensor_copy(Sb16[:], Sloc[:])
                ps_t = psB.tile([CPC, 128], bf16, tag="ps")
                nc.tensor.transpose(ps_t[:], Sb16[:], ident[:])
                S_jd = pers.tile([CPC, 128], fp8)
                nc.vector.tensor_copy(S_jd[:], ps_t[:])
                ccin = dramp.tile([CPC, D], fp8)
                nc.sync.dma_start(ccin[:], S_jd[:])
                ccout = dramp.tile([C, D], fp8)
                if _NO_CC:
                    for r in range(NCORES):
                        nc.sync.dma_start(ccout[CPC * r : CPC * (r + 1), :], ccin[:])
                else:
                    nc.gpsimd.collective_compute(
                        "AllGather", OP.bypass,
                        replica_groups=[list(range(NCORES))],
                        ins=[ccin[:].opt()],
                        outs=[ccout[:].opt()],
                    )
                gsb = pers.tile([128, 4, 128], fp8)
                ccv = ccout.rearrange("(ch c) d -> c ch d", ch=4)
                for ch in range(4):
                    eng = nc.sync if ch % 2 == 0 else nc.scalar
                    eng.dma_start(gsb[:, ch, :], ccv[:, ch, :])

                # independent precompute while the collective runs
                ps_cb = psB.tile([128, CPC], f32, tag="ps")
                nc.tensor.matmul(ps_cb[:], ones1[:], cntrow[:], start=True, stop=True)
                cntbc = pers.tile([128, CPC], f32)
                nc.vector.tensor_copy(cntbc[:], ps_cb[:])
                ps_cb2 = psB.tile([128, CPC], f32, tag="ps")
                nc.tensor.matmul(ps_cb2[:], ones1[:], cm1r[:], start=True, stop=True)
                cm1bc = pers.tile([128, CPC], f32)
                nc.vector.tensor_copy(cm1bc[:], ps_cb2[:])

                # rotated means: m2T = -2*mean, scale folded into prot values.
                # 128-col chunks ride the PE pstate ramp up faster.
                ps_r = psB.tile([128, C], f32, tag="ps")
                for cb in range(4):
                    for ch in range(4):
                        nc.tensor.matmul(ps_r[:, 128 * cb : 128 * (cb + 1)],
                                         gsb[:, ch, :],
                                         prot[:, ch, 128 * cb : 128 * (cb + 1)],
                                         start=(ch == 0), stop=(ch == 3))
                m2T = pers.tile([128, C], bf16)
                nc.scalar.copy(m2T[:], ps_r[:])
                msqb = pers.tile([128, C], bf16)
                nc.vector.tensor_mul(msqb[:], m2T[:], m2T[:])
                ps_c = psB.tile([1, C], f32, tag="ps")
                nc.tensor.matmul(ps_c[:], col025[:], msqb[:], start=True, stop=True)
                nc.vector.tensor_copy(rhs2[0:1, :], ps_c[:])

                # everything below is off the phase-C critical chain
                c2f = pers.tile([1, C], f32)
                nc.scalar.copy(c2f[:], ps_c[:])
                c2rep = pers.tile([1, NT], f32)
                nc.vector.tensor_copy(c2rep[0:1, 0:NT:2], c2f[0:1, 0:CPC])
                nc.vector.tensor_copy(c2rep[0:1, 1:NT:2], c2f[0:1, 0:CPC])
                c2cols = pers.tile([128, NT], f32)
                nc.gpsimd.partition_broadcast(c2cols[:], c2rep[:])
                xc2sum = pers.tile([128, NT], f32)
                nc.vector.tensor_add(xc2sum[:], x2c[:], c2cols[:])
                S2 = pers.tile([128, NT], f32)
                nc.vector.tensor_mul(S2[:], c2cols[:], cnt2c[:])
                apt = pers.tile([128, NT], f32)
                nc.vector.tensor_mul(apt[:], aa[:], S2[:])
                nc.vector.tensor_add(apt[:], apt[:], t2[:])

                # std loss (independent of phase C)
                sq = pers.tile([128, CPC], f32)
                nc.vector.tensor_mul(sq[:], msqb[:, 0:CPC], cntbc[:])
                nc.vector.tensor_scalar(sq[:], sq[:], -0.25, None, op0=OP.mult)
                nc.vector.tensor_add(sq[:], sq[:], SQloc[:])
                nc.vector.tensor_mul(sq[:], sq[:], cm1bc[:])
                nc.vector.tensor_scalar(sq[:], sq[:], 0.0, None, op0=OP.max)
                stdv = pers.tile([128, CPC], f32)
                nc.scalar.activation(stdv[:], sq[:], ACTF.Sqrt)
                nc.vector.tensor_scalar(stdv[:], stdv[:], -1.0, None, op0=OP.add)
                nc.vector.tensor_mul(stdv[:], stdv[:], stdv[:])
                red = pers.tile([128, 4], f32)
                nc.vector.tensor_reduce(red[:, 2:3], stdv[:], axis=AX.X, op=OP.add)
                nc.vector.memset(red[:, 3:4], 0.0)

            # ---------- Phase C: distance tiles ----------
            sumc = pers.tile([128, NT], f32)
            ownc = pers.tile([128, NT], f32)
            min16 = pers.tile([128, NT], fp16)
            MINW = 64  # per-tile min tree stops at this width
            with tc.tile_pool(name="psC", bufs=2, space="PSUM") as psC:
                for g in range(NGRP):
                    d16 = distp.tile([128, GRP * C], fp16)
                    ps4 = psC.tile([128, GRP * C], f32, tag="ps")
                    for q in range(GRP):
                        t = GRP * g + q
                        nc.tensor.matmul(ps4[:, C * q : C * (q + 1)],
                                         xsl(t),
                                         m2T[:], start=True, stop=False)
                        nc.tensor.matmul(ps4[:, C * q : C * (q + 1)],
                                         aug2[:, 128 * t : 128 * (t + 1)],
                                         rhs2[:], start=False, stop=True)
                    nc.scalar.activation(d16[:], ps4[:], ACTF.Sqrt)
                    for q in range(GRP):
                        t = GRP * g + q
                        nc.vector.tensor_scalar(
                            d16[:, C * q : C * (q + 1)], d16[:, C * q : C * (q + 1)],
                            0.0, 0.0, op0=OP.add, op1=OP.add,
                            accum_out=sumc[:, t : t + 1])
                    # own cols of tiles 4g..4g+3 sit at stride-C pairs:
                    # [2g, C+2g] and [2C+2g+1, 3C+2g+1]
                    o0 = 2 * g
                    o1 = 2 * C + 2 * g + 1
                    nc.gpsimd.tensor_copy(
                        ownc[:, 4 * g : 4 * g + 2],
                        d16[:, o0 : o0 + C + 1 : C])
                    nc.gpsimd.tensor_copy(
                        ownc[:, 4 * g + 2 : 4 * g + 4],
                        d16[:, o1 : o1 + C + 1 : C])
                    nc.gpsimd.memset(d16[:, o0 : o0 + C + 1 : C], BIG16)
                    nc.gpsimd.memset(d16[:, o1 : o1 + C + 1 : C], BIG16)
                    # binary TT-min tree per tile, in place: width 512 -> MINW
                    w = C
                    v = d16.rearrange("p (q c) -> p q c", q=GRP)
                    while w > MINW:
                        h = w // 2
                        nc.vector.tensor_tensor(
                            v[:, :, 0:h], v[:, :, 0:h], v[:, :, h:w], op=OP.min)
                        w = h
                    nc.vector.tensor_reduce(
                        min16[:, GRP * g : GRP * (g + 1)],
                        v[:, :, 0:MINW], axis=AX.X, op=OP.min)

            # ---------- Phase D: per-node algebra ----------
            if True:
                own2 = pers.tile([128, NT], f32)
                nc.vector.tensor_mul(own2[:], ownc[:], ownc[:])
                tx = pers.tile([128, NT], f32)
                nc.vector.tensor_sub(tx[:], xc2sum[:], own2[:])
                xS = pers.tile([128, NT], f32)
                nc.vector.tensor_mul(xS[:], tx[:], ca2[:])
                pos2 = pers.tile([128, NT], f32)
                nc.vector.tensor_sub(pos2[:], apt[:], xS[:])
                nc.vector.tensor_scalar(pos2[:], pos2[:], 0.0, None, op0=OP.max)
                pos = pers.tile([128, NT], f32)
                nc.scalar.activation(pos[:], pos2[:], ACTF.Sqrt)
                minc = pers.tile([128, NT], f32)
                nc.gpsimd.tensor_copy(minc[:], min16[:])
                tm = pers.tile([128, NT], f32)
                nc.vector.scalar_tensor_tensor(
                    tm[:], sumc[:], -1.0 / (C - 1), pos[:], op0=OP.mult, op1=OP.add)
                nc.vector.scalar_tensor_tensor(
                    tm[:], ownc[:], 1.0 / (C - 1), tm[:], op0=OP.mult, op1=OP.add)
                nc.vector.tensor_scalar(tm[:], tm[:], ALPHA, 0.0, op0=OP.add, op1=OP.max)
                tn = pers.tile([128, NT], f32)
                nc.vector.tensor_sub(tn[:], pos[:], minc[:])
                nc.vector.tensor_scalar(tn[:], tn[:], ALPHA, 0.0, op0=OP.add, op1=OP.max)
                nc.vector.tensor_reduce(red[:, 0:1], tm[:], axis=AX.X, op=OP.add)
                nc.vector.tensor_reduce(red[:, 1:2], tn[:], axis=AX.X, op=OP.add)
                nc.sync.dma_start(out_d[:], red[:])

    nc.compile()
    return nc


def _host_prep(node_features, community_belong_list):
    x = np.asarray(node_features)
    if x.ndim == 3:
        x = x[0]
    x = np.ascontiguousarray(x, dtype=np.float32)  # [N, D]
    cbl = np.asarray(community_belong_list).astype(np.int64)
    N = x.shape[0]
    comm = np.empty(N, np.int64)
    comm[cbl[0]] = cbl[1]
    counts = np.bincount(comm, minlength=C).astype(np.int64)
    assert counts.min() >= 2, f"community with <2 nodes: {counts.min()}"
    assert counts.max() <= KSLOT, f"community overflow: {counts.max()} > {KSLOT}"

    order = np.argsort(comm, kind="stable")
    comm_sorted = comm[order]
    cstart = np.zeros(C + 1, np.int64)
    np.cumsum(counts, out=cstart[1:])
    ranks = np.arange(N) - cstart[comm_sorted]
    slot_global = comm_sorted * KSLOT + ranks

    X_pad = np.zeros((C * KSLOT, D), np.float32)
    X_pad[slot_global] = x[order]
    x2 = (x.astype(np.float64) ** 2).sum(1).astype(np.float32)
    x2_pad = np.zeros(C * KSLOT, np.float32)
    x2_pad[slot_global] = x2[order]
    valid_pad = np.zeros(C * KSLOT, np.float32)
    valid_pad[slot_global] = 1.0

    countsf = counts.astype(np.float32)
    ident = np.eye(128, dtype=ml_dtypes.bfloat16)

    in_maps = []
    for k in range(NCORES):
        lo = k * SLOTS
        hi = lo + SLOTS
        xTk = np.ascontiguousarray(X_pad[lo:hi].T).astype(ml_dtypes.bfloat16)
        x2k = np.ascontiguousarray(x2_pad[lo:hi].reshape(NT, 128).T)
        vk = np.ascontiguousarray(valid_pad[lo:hi].reshape(NT, 128).T)
        ccore = counts[k * CPC : (k + 1) * CPC].astype(np.float32)
        cnt_col = np.repeat(ccore, 2)
        acol = np.broadcast_to(1.0 / (cnt_col - 1.0), (128, NT)).astype(np.float32)
        cnt2c = np.broadcast_to(cnt_col * cnt_col, (128, NT)).astype(np.float32).copy()
        aak = (acol * acol).copy()
        ca2k = np.broadcast_to(cnt_col, (128, NT)) * acol * (1.0 + acol)
        t2k = ((1.0 + acol) ** 2 * x2k).astype(np.float32)
        t2k[vk == 0.0] = NEGBIG
        cntrow = ccore.reshape(1, CPC)
        cm1r = (1.0 / (ccore - 1.0)).reshape(1, CPC).astype(np.float32)
        src = (k * CPC + np.arange(C)) % C
        prot = np.zeros((C, C), np.float32)
        prot[src, np.arange(C)] = -2.0 / countsf[src]
        prot = prot.astype(mybir.dt.np(mybir.dt.float8e4))
        aug2 = np.ones((2, SLOTS), np.float32)
        aug2[1] = x2_pad[lo:hi]
        m = {
            "xT": xTk,
            "aug2": aug2.astype(ml_dtypes.bfloat16),
            "x2c": x2k,
            "ca2": ca2k.astype(np.float32),
            "cnt2c": cnt2c,
            "aa": aak,
            "t2": t2k,
            "cntrow": cntrow.astype(np.float32),
            "cm1r": cm1r,
            "prot": prot,
            "ident": ident,
        }
        in_maps.append(m)
    return in_maps


def kernel(node_features, community_belong_list):
    global _PROG
    in_maps = _host_prep(node_features, community_belong_list)
    if _PROG is None:
        _PROG = _build_program()
    r = run_bass_kernel_spmd(_PROG, in_maps, list(range(NCORES)))
    tm = sum(float(r.results[k]["out"][:, 0].sum()) for k in range(NCORES))
    tn = sum(float(r.results[k]["out"][:, 1].sum()) for k in range(NCORES))
    sd = sum(float(r.results[k]["out"][:, 2].sum()) for k in range(NCORES))
    kernel._last_results = r
    out = np.array([tm / N_NODES, tn / N_NODES, sd / (C * D)], dtype=np.float32)
    return out
